# revision 5
# baseline (speedup 1.0000x reference)
"""Direct-Form-II biquad (order-2 IIR) over [B=64, T=262144, 1] on 8 trn2 cores.

Algorithm
---------
The recurrence
    y[t] = b0 x[t] + b1 x[t-1] + b2 x[t-2] - a1 y[t-1] - a2 y[t-2]
is a linear time-invariant filter whose impulse response g decays
geometrically (|poles| < 1 for the sampled coefficients), so to fp32
precision the IIR equals a short FIR: y = conv(x, g[:K]).

On device the FIR is computed with the tensor engine in overlap-save form.
Per sequence, x is laid out in SBUF as [128 partitions, 2048] with partition
p holding x[p*2048 : (p+1)*2048] (contiguous DMA). Each 128x128 tile of that
layout holds 128 chunks (partitions = chunk index c = p*16 + f1, free =
within-chunk time j). Tiles are PE-transposed so j lands on partitions, then
one matmul per tile, with the transposed tile as the stationary operand and a
fused [A^T | B^T] Toeplitz coefficient block as the moving operand, produces
the within-chunk FIR term (A-half) and the spill-over into the next chunk
(B-half). ys[f1] = A(f1) + B(f1-1) is assembled during PSUM evacuation:
an A-copy (ACT/DVE) plus a read-modify-write B-add (DVE) — PSUM has a single
DVE read port, so the two PSUM halves are never read by one instruction.

Sharding: pure data parallelism, batch 64 -> 8 sequences per core.
"""

import os
from contextlib import ExitStack

import numpy as np

_B, _T = 64, 262144
_NCORES = 8
_S = _B // _NCORES          # sequences per core
_P = 128                    # partitions / chunk length
_F = _T // _P               # 2048 free columns per sequence
_NT = _F // _P              # 16 tiles per sequence

# 'fp32'  : exact fp32 matmuls (4 cycles/row on PE)
# 'f32r'  : rounded fp32 (12-bit mantissa) matmuls at full PE rate
_MODE = os.environ.get("BIQUAD_MODE", "fp32")

_runner_cache = {}


def _impulse_response(b0, b1, b2, a1, a2, n):
    """Float64 impulse response of the reference recurrence."""
    g = np.zeros(n, dtype=np.float64)
    v0 = 0.0
    v1 = 0.0
    for t in range(n):
        xt = 1.0 if t == 0 else 0.0
        out = xt * b0 + v0
        v0_new = xt * b1 + v1 - out * a1
        v1_new = xt * b2 - out * a2
        v0, v1 = v0_new, v1_new
        g[t] = out
    return g


def _coef_block(g, kb):
    """[128, 128 + kb] moving operand: columns = output offset i.

    A^T[j, i] = g[i - j]          (within-chunk taps, i in [0,128))
    B^T[j, i] = g[i + 128 - j]    (taps reaching one chunk back, i in [0,kb))
    """
    K = len(g)
    A = np.zeros((_P, _P), dtype=np.float64)
    Bm = np.zeros((_P, kb), dtype=np.float64)
    for j in range(_P):
        for i in range(_P):
            if 0 <= i - j < K:
                A[j, i] = g[i - j]
        for i in range(kb):
            k = i + _P - j
            if 0 <= k < K:
                Bm[j, i] = g[k]
    return np.concatenate([A, Bm], axis=1).astype(np.float32)


def _build_program(mode, kb):
    from concourse import bacc, mybir, tile

    nc = bacc.Bacc("TRN2", target_bir_lowering=False, debug=False)
    f32 = mybir.dt.float32
    cdt = mybir.dt.float32r if mode == "f32r" else f32

    NC = _P + kb                      # moving operand width
    SLOT = 256 if NC <= 256 else 512  # psum slot stride (bank-crossing safe)
    x_d = nc.dram_tensor("x", [_S, _P, _F], f32, kind="ExternalInput")
    coef_d = nc.dram_tensor("coef", [_P, NC], f32, kind="ExternalInput")
    id_d = nc.dram_tensor("ident", [_P, _P], f32, kind="ExternalInput")
    y_d = nc.dram_tensor("y", [_S, _P, _F], f32, kind="ExternalOutput")

    XT_SLOTS = _NT + 1                # 16 transposed tiles + shifted m1 tile

    with tile.TileContext(nc) as tc, ExitStack() as ctx:
        cpool = ctx.enter_context(tc.tile_pool(name="consts", bufs=1))
        xpool = ctx.enter_context(tc.tile_pool(name="xin", bufs=2))
        xtpool = ctx.enter_context(tc.tile_pool(name="xt", bufs=2))
        ypool = ctx.enter_context(tc.tile_pool(name="yout", bufs=2))
        ptp = ctx.enter_context(tc.tile_pool(name="pt", bufs=2, space="PSUM"))
        pyp = ctx.enter_context(tc.tile_pool(name="py", bufs=3, space="PSUM"))

        id_sb = cpool.tile([_P, _P], f32)
        nc.sync.dma_start(id_sb[:], id_d.ap())
        coef_sb = cpool.tile([_P, NC], f32)
        nc.sync.dma_start(coef_sb[:], coef_d.ap())
        if mode == "f32r":
            coef_c = cpool.tile([_P, NC], cdt)
            nc.vector.tensor_copy(coef_c[:], coef_sb[:])
        else:
            coef_c = coef_sb

        for s in range(_S):
            # --- load x[s] as [128, 2048], partition p = x[p*2048 + f] ---
            xs = xpool.tile([_P, _F], f32)
            for q in range(4):
                nc.sync.dma_start(
                    xs[:, q * 512 : (q + 1) * 512],
                    x_d.ap()[s][:, q * 512 : (q + 1) * 512],
                )

            # --- PE transposes, 4 per PSUM bank, ACT evacuation ---
            # Tile 15 is transposed first so the m1 boundary tile (and its
            # matmul) can run early, keeping all PSUM pairs short-lived.
            perm = [15] + list(range(15))
            xt = xtpool.tile([_P, XT_SLOTS * _P], cdt)
            xt32 = xt[:].bitcast(f32)
            for gidx in range(4):
                ptile = ptp.tile([_P, 512], f32)
                grp = perm[4 * gidx : 4 * gidx + 4]
                for q, f1 in enumerate(grp):
                    nc.tensor.transpose(
                        ptile[:, q * _P : (q + 1) * _P],
                        xs[:, f1 * _P : (f1 + 1) * _P],
                        id_sb[:],
                    )
                # copy contiguous runs of the permuted group into xt
                q0 = 0
                while q0 < 4:
                    q1 = q0 + 1
                    while q1 < 4 and grp[q1] == grp[q1 - 1] + 1:
                        q1 += 1
                    nc.scalar.copy(
                        xt[:, grp[q0] * _P : (grp[q0] + q1 - q0) * _P],
                        ptile[:, q0 * _P : q1 * _P],
                    )
                    q0 = q1

            # --- m1 boundary tile: m1[col p] = tile15[col p-1], col 0 = 0 ---
            m1 = _NT * _P
            nc.gpsimd.memset(xt32[:, m1 : m1 + 1], 0.0)
            nc.gpsimd.tensor_copy(
                xt32[:, m1 + 1 : m1 + _P],
                xt32[:, 15 * _P : 16 * _P - 1],
            )

            # --- matmuls (fused [A|B] moving operand) + batched evacuation ---
            # Groups of 4 matmuls per PSUM tile:
            #   G0=[m1,t0,t1,t2] G1=[t3..6] G2=[t7..10] G3=[t11..14] G4=[t15]
            # ys[f1] = A-half(f1) + B-half(f1-1); m1's B-half feeds ys[0].
            ys = ypool.tile([_P, _F], f32)
            groups = [[_NT, 0, 1, 2], [3, 4, 5, 6], [7, 8, 9, 10],
                      [11, 12, 13, 14], [15]]
            ptiles = []

            def mm_group(gi):
                grp = groups[gi]
                pt_ = pyp.tile([_P, 4 * SLOT], f32, tag="py")
                for k, idx in enumerate(grp):
                    nc.tensor.matmul(
                        pt_[:, k * SLOT : k * SLOT + NC],
                        xt[:, idx * _P : (idx + 1) * _P],
                        coef_c[:],
                        start=True,
                        stop=True,
                    )
                ptiles.append(pt_)

            def a_copy(gi, eng):
                # copy A-halves of the group's data tiles (skip m1) into ys
                grp = groups[gi]
                pt_ = ptiles[gi]
                k0 = 1 if gi == 0 else 0
                n = len(grp) - k0
                t0 = grp[k0]
                src = pt_[:, k0 * SLOT : (k0 + n) * SLOT].rearrange(
                    "p (n w) -> p n w", w=SLOT
                )[:, :, 0:_P]
                dst = ys[:, t0 * _P : (t0 + n) * _P].rearrange(
                    "p (n w) -> p n w", w=_P
                )
                eng(dst, src)

            def b_add(gi):
                # ys[f1] += B-half(f1-1): group gi's slots feed the next tiles
                grp = groups[gi]
                if grp[-1] == 15:
                    grp = grp[:-1]      # tile 15's B-half is discarded
                    if not grp:
                        return
                pt_ = ptiles[gi]
                n = len(grp)
                tdst = 0 if gi == 0 else groups[gi][0] + 1
                src = pt_[:, 0 : n * SLOT].rearrange(
                    "p (n w) -> p n w", w=SLOT
                )[:, :, _P : _P + kb]
                dst = ys[:, tdst * _P : (tdst + n) * _P].rearrange(
                    "p (n w) -> p n w", w=_P
                )[:, :, 0:kb]
                nc.vector.tensor_add(dst, src, dst)

            act_copy = nc.scalar.copy
            dve_copy = nc.vector.tensor_copy
            # A-copy engine per group: balance ACT (also does Xt copies)
            a_eng = [dve_copy, act_copy, dve_copy, act_copy, dve_copy]

            mm_group(0)
            a_copy(0, a_eng[0])
            for gi in range(1, 5):
                mm_group(gi)
                a_copy(gi, a_eng[gi])
                b_add(gi - 1)
            b_add(4)

            nc.gpsimd.dma_start(y_d.ap()[s], ys[:])

    nc.compile()
    return nc


def _make_runner(mode, kb):
    """Compile the bass program and wrap it in a cached shard_map'd jit."""
    import jax
    import numpy as _np
    from jax.sharding import Mesh, PartitionSpec
    from jax.experimental.shard_map import shard_map
    from concourse import bass2jax, mybir

    nc = _build_program(mode, kb)

    if os.environ.get("BIQUAD_SIM") == "1":
        def run_sim(x_all, coef):
            from concourse import bass_interp
            y_all = np.zeros_like(x_all)
            ident = np.eye(_P, dtype=np.float32)
            ncs = int(os.environ.get("BIQUAD_SIM_CORES", str(_NCORES)))
            for c in range(ncs):
                sim = bass_interp.CoreSim(nc)
                sim.tensor("x")[:] = x_all[c * _S : (c + 1) * _S]
                sim.tensor("coef")[:] = coef
                sim.tensor("ident")[:] = ident
                sim.simulate()
                y_all[c * _S : (c + 1) * _S] = sim.tensor("y")
            return y_all
        return run_sim

    bass2jax.install_neuronx_cc_hook()

    partition_name = (
        nc.partition_id_tensor.name if nc.partition_id_tensor else None
    )
    in_names, out_names, out_avals = [], [], []
    for alloc in nc.m.functions[0].allocations:
        if not isinstance(alloc, mybir.MemoryLocationSet):
            continue
        name = alloc.memorylocations[0].name
        if alloc.kind == "ExternalInput":
            if name != partition_name:
                in_names.append(name)
        elif alloc.kind == "ExternalOutput":
            out_names.append(name)
            out_avals.append(
                jax.core.ShapedArray(
                    tuple(alloc.tensor_shape), mybir.dt.np(alloc.dtype)
                )
            )
    n_params = len(in_names)
    in_names.extend(out_names)
    if partition_name is not None:
        in_names.append(partition_name)

    def _body(*args):
        operands = list(args)
        if partition_name is not None:
            operands.append(bass2jax.partition_id_tensor())
        outs = bass2jax._bass_exec_p.bind(
            *operands,
            out_avals=tuple(out_avals),
            in_names=tuple(in_names),
            out_names=tuple(out_names),
            lowering_input_output_aliases=(),
            sim_require_finite=True,
            sim_require_nnan=True,
            nc=nc,
        )
        return tuple(outs)

    devices = jax.devices()[:_NCORES]
    mesh = Mesh(_np.asarray(devices), ("core",))
    n_outs = len(out_names)
    in_specs = (PartitionSpec("core"),) * (n_params + n_outs)
    out_specs = (PartitionSpec("core"),) * n_outs
    sharded = jax.jit(
        shard_map(
            _body, mesh=mesh, in_specs=in_specs, out_specs=out_specs,
            check_rep=False,
        ),
        keep_unused=True,
    )

    name_to_idx = {n: i for i, n in enumerate(in_names[:n_params])}
    ident = np.eye(_P, dtype=np.float32)

    def run_hw(x_all, coef):
        # x_all: [64, 128, 2048] fp32; returns y_all same shape
        per_core_ins = {
            "x": x_all.reshape(_NCORES * _S, _P, _F),
            "coef": np.concatenate([coef] * _NCORES, axis=0),
            "ident": np.concatenate([ident] * _NCORES, axis=0),
        }
        args = [None] * n_params
        for n, i in name_to_idx.items():
            args[i] = per_core_ins[n]
        zeros = [
            np.zeros((_NCORES * a.shape[0], *a.shape[1:]), a.dtype)
            for a in out_avals
        ]
        outs = sharded(*args, *zeros)
        y_idx = out_names.index("y")
        return np.asarray(outs[y_idx]).reshape(_B, _P, _F)

    run_hw.sharded = sharded
    run_hw.meta = (in_names, out_names, out_avals, n_params, name_to_idx, ident)
    return run_hw


def _get_runner(mode, kb):
    key = (mode, kb, os.environ.get("BIQUAD_SIM") == "1")
    if key not in _runner_cache:
        _runner_cache[key] = _make_runner(mode, kb)
    return _runner_cache[key]


def _prepare(b0, b1, b2, a1, a2):
    """Impulse response, truncation length, coefficient block."""
    g = _impulse_response(b0, b1, b2, a1, a2, 2 * _P)
    mag = np.abs(g)
    scale = mag.max() + 1e-300
    sig = np.nonzero(mag > 1e-9 * scale)[0]
    K = int(sig[-1]) + 1 if len(sig) else 1
    if K > _P:
        raise ValueError(
            f"impulse response needs {K} taps (> {_P}); filter too close "
            "to instability for the truncated-FIR kernel"
        )
    kb = max(32, ((K + 31) // 32) * 32)   # B-half width, 32-col aligned
    if _MODE == "f32r":
        kb = _P                            # keep N >= 256 for full-rate f32r
    coef = _coef_block(g[: _P + kb], kb)
    return coef, kb


def kernel(x, b0, b1, b2, a1, a2):
    assert x.shape == (_B, _T, 1), x.shape
    coef, kb = _prepare(
        float(b0[0]), float(b1[0]), float(b2[0]), float(a1[0]), float(a2[0])
    )
    run = _get_runner(_MODE, kb)
    x_all = np.ascontiguousarray(x, dtype=np.float32).reshape(_B, _P, _F)
    y_all = run(x_all, coef)
    return y_all.reshape(_B, _T, 1)


# revision 8
# speedup vs baseline: 1.0268x; 1.0268x over previous
"""Direct-Form-II biquad (order-2 IIR) over [B=64, T=262144, 1] on 8 trn2 cores.

Algorithm
---------
The recurrence
    y[t] = b0 x[t] + b1 x[t-1] + b2 x[t-2] - a1 y[t-1] - a2 y[t-2]
is a linear time-invariant filter whose impulse response g decays
geometrically (|poles| < 1 for the sampled coefficients), so to fp32
precision the IIR equals a short FIR: y = conv(x, g[:K]).

On device the FIR is computed with the tensor engine in overlap-save form.
Per sequence, x is laid out in SBUF as [128 partitions, 2048] with partition
p holding x[p*2048 : (p+1)*2048] (contiguous DMA). Each 128x128 tile of that
layout holds 128 chunks (partitions = chunk index c = p*16 + f1, free =
within-chunk time j). Tiles are PE-transposed so j lands on partitions, then
one matmul per tile, with the transposed tile as the stationary operand and a
fused [A^T | B^T] Toeplitz coefficient block as the moving operand, produces
the within-chunk FIR term (A-half) and the spill-over into the next chunk
(B-half). ys[f1] = A(f1) + B(f1-1) is assembled during PSUM evacuation:
an A-copy (ACT/DVE) plus a read-modify-write B-add (DVE) — PSUM has a single
DVE read port, so the two PSUM halves are never read by one instruction.

Sharding: pure data parallelism, batch 64 -> 8 sequences per core.
"""

import os
from contextlib import ExitStack

import numpy as np

_B, _T = 64, 262144
_NCORES = 8
_S = _B // _NCORES          # sequences per core
_P = 128                    # partitions / chunk length
_F = _T // _P               # 2048 free columns per sequence
_NT = _F // _P              # 16 tiles per sequence

# 'fp32'  : exact fp32 matmuls (4 cycles/row on PE)
# 'f32r'  : rounded fp32 (12-bit mantissa) matmuls at full PE rate
_MODE = os.environ.get("BIQUAD_MODE", "fp32")

_runner_cache = {}


def _impulse_response(b0, b1, b2, a1, a2, n):
    """Float64 impulse response of the reference recurrence."""
    g = np.zeros(n, dtype=np.float64)
    v0 = 0.0
    v1 = 0.0
    for t in range(n):
        xt = 1.0 if t == 0 else 0.0
        out = xt * b0 + v0
        v0_new = xt * b1 + v1 - out * a1
        v1_new = xt * b2 - out * a2
        v0, v1 = v0_new, v1_new
        g[t] = out
    return g


def _coef_block(g, kb):
    """[128, 128 + kb] moving operand: columns = output offset i.

    A^T[j, i] = g[i - j]          (within-chunk taps, i in [0,128))
    B^T[j, i] = g[i + 128 - j]    (taps reaching one chunk back, i in [0,kb))
    """
    K = len(g)
    A = np.zeros((_P, _P), dtype=np.float64)
    Bm = np.zeros((_P, kb), dtype=np.float64)
    for j in range(_P):
        for i in range(_P):
            if 0 <= i - j < K:
                A[j, i] = g[i - j]
        for i in range(kb):
            k = i + _P - j
            if 0 <= k < K:
                Bm[j, i] = g[k]
    return np.concatenate([A, Bm], axis=1).astype(np.float32)


def _build_program(mode, kb):
    from concourse import bacc, mybir, tile

    nc = bacc.Bacc("TRN2", target_bir_lowering=False, debug=False)
    f32 = mybir.dt.float32
    cdt = mybir.dt.float32r if mode == "f32r" else f32

    NC = _P + kb                      # moving operand width
    SLOT = 256 if NC <= 256 else 512  # psum slot stride (bank-crossing safe)
    x_d = nc.dram_tensor("x", [_S, _P, _F], f32, kind="ExternalInput")
    coef_d = nc.dram_tensor("coef", [_P, NC], f32, kind="ExternalInput")
    id_d = nc.dram_tensor("ident", [_P, _P], f32, kind="ExternalInput")
    y_d = nc.dram_tensor("y", [_S, _P, _F], f32, kind="ExternalOutput")

    XT_SLOTS = _NT + 1                # 16 transposed tiles + shifted m1 tile

    with tile.TileContext(nc) as tc, ExitStack() as ctx:
        cpool = ctx.enter_context(tc.tile_pool(name="consts", bufs=1))
        xpool = ctx.enter_context(tc.tile_pool(name="xin", bufs=2))
        xtpool = ctx.enter_context(tc.tile_pool(name="xt", bufs=2))
        ypool = ctx.enter_context(tc.tile_pool(name="yout", bufs=2))
        ptp = ctx.enter_context(tc.tile_pool(name="pt", bufs=2, space="PSUM"))
        pyp = ctx.enter_context(tc.tile_pool(name="py", bufs=3, space="PSUM"))

        id_sb = cpool.tile([_P, _P], f32)
        nc.sync.dma_start(id_sb[:], id_d.ap())
        coef_sb = cpool.tile([_P, NC], f32)
        nc.sync.dma_start(coef_sb[:], coef_d.ap())
        if mode == "f32r":
            coef_c = cpool.tile([_P, NC], cdt)
            nc.vector.tensor_copy(coef_c[:], coef_sb[:])
        else:
            coef_c = coef_sb

        for s in range(_S):
            # --- load x[s] as [128, 2048], partition p = x[p*2048 + f] ---
            xs = xpool.tile([_P, _F], f32)
            for q in range(4):
                nc.sync.dma_start(
                    xs[:, q * 512 : (q + 1) * 512],
                    x_d.ap()[s][:, q * 512 : (q + 1) * 512],
                )

            # --- PE transposes, 4 per PSUM bank, ACT evacuation ---
            # Tile 15 is transposed first so the m1 boundary tile (and its
            # matmul) can run early, keeping all PSUM pairs short-lived.
            perm = [15] + list(range(15))
            xt = xtpool.tile([_P, XT_SLOTS * _P], cdt)
            xt32 = xt[:].bitcast(f32)
            for gidx in range(4):
                ptile = ptp.tile([_P, 512], f32)
                grp = perm[4 * gidx : 4 * gidx + 4]
                for q, f1 in enumerate(grp):
                    nc.tensor.transpose(
                        ptile[:, q * _P : (q + 1) * _P],
                        xs[:, f1 * _P : (f1 + 1) * _P],
                        id_sb[:],
                    )
                # copy contiguous runs of the permuted group into xt
                q0 = 0
                while q0 < 4:
                    q1 = q0 + 1
                    while q1 < 4 and grp[q1] == grp[q1 - 1] + 1:
                        q1 += 1
                    nc.scalar.copy(
                        xt[:, grp[q0] * _P : (grp[q0] + q1 - q0) * _P],
                        ptile[:, q0 * _P : q1 * _P],
                    )
                    q0 = q1

            # --- m1 boundary tile: m1[col p] = tile15[col p-1], col 0 = 0 ---
            m1 = _NT * _P
            nc.gpsimd.memset(xt32[:, m1 : m1 + 1], 0.0)
            nc.gpsimd.tensor_copy(
                xt32[:, m1 + 1 : m1 + _P],
                xt32[:, 15 * _P : 16 * _P - 1],
            )

            # --- matmuls (fused [A|B] moving operand) + batched evacuation ---
            # Groups of 4 matmuls per PSUM tile:
            #   G0=[m1,t0,t1,t2] G1=[t3..6] G2=[t7..10] G3=[t11..14] G4=[t15]
            # ys[f1] = A-half(f1) + B-half(f1-1); m1's B-half feeds ys[0].
            ys = ypool.tile([_P, _F], f32)
            groups = [[_NT, 0, 1, 2], [3, 4, 5, 6], [7, 8, 9, 10],
                      [11, 12, 13, 14], [15]]
            ptiles = []

            def mm_group(gi):
                grp = groups[gi]
                pt_ = pyp.tile([_P, 4 * SLOT], f32, tag="py")
                for k, idx in enumerate(grp):
                    nc.tensor.matmul(
                        pt_[:, k * SLOT : k * SLOT + NC],
                        xt[:, idx * _P : (idx + 1) * _P],
                        coef_c[:],
                        start=True,
                        stop=True,
                    )
                ptiles.append(pt_)

            def a_copy(gi, eng):
                # copy A-halves of the group's data tiles (skip m1) into ys
                grp = groups[gi]
                pt_ = ptiles[gi]
                k0 = 1 if gi == 0 else 0
                n = len(grp) - k0
                t0 = grp[k0]
                src = pt_[:, k0 * SLOT : (k0 + n) * SLOT].rearrange(
                    "p (n w) -> p n w", w=SLOT
                )[:, :, 0:_P]
                dst = ys[:, t0 * _P : (t0 + n) * _P].rearrange(
                    "p (n w) -> p n w", w=_P
                )
                eng(dst, src)

            def b_add(gi):
                # ys[f1] += B-half(f1-1): group gi's slots feed the next tiles
                grp = groups[gi]
                if grp[-1] == 15:
                    grp = grp[:-1]      # tile 15's B-half is discarded
                    if not grp:
                        return
                pt_ = ptiles[gi]
                n = len(grp)
                tdst = 0 if gi == 0 else groups[gi][0] + 1
                src = pt_[:, 0 : n * SLOT].rearrange(
                    "p (n w) -> p n w", w=SLOT
                )[:, :, _P : _P + kb]
                dst = ys[:, tdst * _P : (tdst + n) * _P].rearrange(
                    "p (n w) -> p n w", w=_P
                )[:, :, 0:kb]
                nc.vector.tensor_add(dst, src, dst)

            act_copy = nc.scalar.copy
            dve_copy = nc.vector.tensor_copy
            # A-copy engine per group: balance ACT (also does Xt copies)
            a_eng = [dve_copy, act_copy, dve_copy, act_copy, dve_copy]

            mm_group(0)
            a_copy(0, a_eng[0])
            for gi in range(1, 5):
                mm_group(gi)
                a_copy(gi, a_eng[gi])
                b_add(gi - 1)
            b_add(4)

            nc.gpsimd.dma_start(y_d.ap()[s], ys[:])

    nc.compile()
    return nc


def _make_runner(mode, kb):
    """Compile the bass program and wrap it in a cached shard_map'd jit."""
    import jax
    import numpy as _np
    from jax.sharding import Mesh, PartitionSpec
    from jax.experimental.shard_map import shard_map
    from concourse import bass2jax, mybir

    nc = _build_program(mode, kb)

    if os.environ.get("BIQUAD_SIM") == "1":
        def run_sim(x_all, coef):
            from concourse import bass_interp
            y_all = np.zeros_like(x_all)
            ident = np.eye(_P, dtype=np.float32)
            ncs = int(os.environ.get("BIQUAD_SIM_CORES", str(_NCORES)))
            for c in range(ncs):
                sim = bass_interp.CoreSim(nc)
                sim.tensor("x")[:] = x_all[c * _S : (c + 1) * _S]
                sim.tensor("coef")[:] = coef
                sim.tensor("ident")[:] = ident
                sim.simulate()
                y_all[c * _S : (c + 1) * _S] = sim.tensor("y")
            return y_all
        return run_sim

    bass2jax.install_neuronx_cc_hook()

    partition_name = (
        nc.partition_id_tensor.name if nc.partition_id_tensor else None
    )
    in_names, out_names, out_avals = [], [], []
    for alloc in nc.m.functions[0].allocations:
        if not isinstance(alloc, mybir.MemoryLocationSet):
            continue
        name = alloc.memorylocations[0].name
        if alloc.kind == "ExternalInput":
            if name != partition_name:
                in_names.append(name)
        elif alloc.kind == "ExternalOutput":
            out_names.append(name)
            out_avals.append(
                jax.core.ShapedArray(
                    tuple(alloc.tensor_shape), mybir.dt.np(alloc.dtype)
                )
            )
    n_params = len(in_names)
    in_names.extend(out_names)
    if partition_name is not None:
        in_names.append(partition_name)

    def _body(*args):
        operands = list(args)
        if partition_name is not None:
            operands.append(bass2jax.partition_id_tensor())
        outs = bass2jax._bass_exec_p.bind(
            *operands,
            out_avals=tuple(out_avals),
            in_names=tuple(in_names),
            out_names=tuple(out_names),
            lowering_input_output_aliases=(),
            sim_require_finite=True,
            sim_require_nnan=True,
            nc=nc,
        )
        return tuple(outs)

    devices = jax.devices()[:_NCORES]
    mesh = Mesh(_np.asarray(devices), ("core",))
    n_outs = len(out_names)
    in_specs = (PartitionSpec("core"),) * (n_params + n_outs)
    out_specs = (PartitionSpec("core"),) * n_outs
    sharded = jax.jit(
        shard_map(
            _body, mesh=mesh, in_specs=in_specs, out_specs=out_specs,
            check_rep=False,
        ),
        keep_unused=True,
    )

    name_to_idx = {n: i for i, n in enumerate(in_names[:n_params])}
    ident = np.eye(_P, dtype=np.float32)

    def run_hw(x_all, coef):
        # x_all: [64, 128, 2048] fp32; returns y_all same shape
        per_core_ins = {
            "x": x_all.reshape(_NCORES * _S, _P, _F),
            "coef": np.concatenate([coef] * _NCORES, axis=0),
            "ident": np.concatenate([ident] * _NCORES, axis=0),
        }
        args = [None] * n_params
        for n, i in name_to_idx.items():
            args[i] = per_core_ins[n]
        zeros = [
            np.zeros((_NCORES * a.shape[0], *a.shape[1:]), a.dtype)
            for a in out_avals
        ]
        outs = sharded(*args, *zeros)
        y_idx = out_names.index("y")
        return np.asarray(outs[y_idx]).reshape(_B, _P, _F)

    run_hw.sharded = sharded
    run_hw.meta = (in_names, out_names, out_avals, n_params, name_to_idx, ident)
    run_hw.nc = nc

    def make_chain():
        """Jit that runs the kernel k (runtime scalar) times back-to-back on
        device, feeding y back as x — for timing (marginal cost per step ≈
        one on-device execution). fori_loop keeps the bass_exec custom call
        appearing exactly once in the module (hook limitation), and a
        runtime k avoids recompiling per chain length."""
        x_idx = name_to_idx["x"]
        y_idx = out_names.index("y")

        def chained(k, *args):
            args = list(args)

            def body(_, x):
                a = list(args)
                a[x_idx] = x
                return _body(*a)[y_idx]

            y = jax.lax.fori_loop(0, k, body, args[x_idx])
            return (y,)

        return jax.jit(
            shard_map(
                chained, mesh=mesh,
                in_specs=(PartitionSpec(),) + in_specs,
                out_specs=(PartitionSpec("core"),),
                check_rep=False,
            ),
            keep_unused=True,
        )

    run_hw.make_chain = make_chain
    return run_hw


def _get_runner(mode, kb):
    key = (mode, kb, os.environ.get("BIQUAD_SIM") == "1")
    if key not in _runner_cache:
        _runner_cache[key] = _make_runner(mode, kb)
    return _runner_cache[key]


def _prepare(b0, b1, b2, a1, a2):
    """Impulse response, truncation length, coefficient block."""
    g = _impulse_response(b0, b1, b2, a1, a2, 2 * _P)
    mag = np.abs(g)
    scale = mag.max() + 1e-300
    sig = np.nonzero(mag > 1e-9 * scale)[0]
    K = int(sig[-1]) + 1 if len(sig) else 1
    if K > _P:
        raise ValueError(
            f"impulse response needs {K} taps (> {_P}); filter too close "
            "to instability for the truncated-FIR kernel"
        )
    kb = max(32, ((K + 31) // 32) * 32)   # B-half width, 32-col aligned
    if _MODE == "f32r":
        kb = _P                            # keep N >= 256 for full-rate f32r
    coef = _coef_block(g[: _P + kb], kb)
    return coef, kb


def kernel(x, b0, b1, b2, a1, a2):
    assert x.shape == (_B, _T, 1), x.shape
    coef, kb = _prepare(
        float(b0[0]), float(b1[0]), float(b2[0]), float(a1[0]), float(a2[0])
    )
    run = _get_runner(_MODE, kb)
    x_all = np.ascontiguousarray(x, dtype=np.float32).reshape(_B, _P, _F)
    y_all = run(x_all, coef)
    return y_all.reshape(_B, _T, 1)


# revision 10
# speedup vs baseline: 337.4262x; 328.6187x over previous
"""Direct-Form-II biquad (order-2 IIR) over [B=64, T=262144, 1] on 8 trn2 cores.

Algorithm
---------
The recurrence
    y[t] = b0 x[t] + b1 x[t-1] + b2 x[t-2] - a1 y[t-1] - a2 y[t-2]
is a linear time-invariant filter whose impulse response g decays
geometrically (|poles| < 1 for the sampled coefficients), so to fp32
precision the IIR equals a short FIR: y = conv(x, g[:K]).

On device the FIR is computed with the tensor engine in overlap-save form.
Per sequence, x is laid out in SBUF as [128 partitions, 2048] with partition
p holding x[p*2048 : (p+1)*2048] (contiguous DMA). Each 128x128 tile of that
layout holds 128 chunks (partitions = chunk index c = p*16 + f1, free =
within-chunk time j). Tiles are PE-transposed so j lands on partitions, then
one matmul per tile, with the transposed tile as the stationary operand and a
fused [A^T | B^T] Toeplitz coefficient block as the moving operand, produces
the within-chunk FIR term (A-half) and the spill-over into the next chunk
(B-half). ys[f1] = A(f1) + B(f1-1) is assembled during PSUM evacuation:
an A-copy (ACT/DVE) plus a read-modify-write B-add (DVE) — PSUM has a single
DVE read port, so the two PSUM halves are never read by one instruction.

Sharding: pure data parallelism, batch 64 -> 8 sequences per core.
"""

import os
from contextlib import ExitStack

import numpy as np

_B, _T = 64, 262144
_NCORES = 8
_S = _B // _NCORES          # sequences per core
_P = 128                    # partitions / chunk length
_F = _T // _P               # 2048 free columns per sequence
_NT = _F // _P              # 16 tiles per sequence

# 'fp32'  : exact fp32 matmuls (4 cycles/row on PE)
# 'f32r'  : rounded fp32 (12-bit mantissa) matmuls at full PE rate
_MODE = os.environ.get("BIQUAD_MODE", "fp32")

_runner_cache = {}


def _impulse_response(b0, b1, b2, a1, a2, n):
    """Float64 impulse response of the reference recurrence."""
    g = np.zeros(n, dtype=np.float64)
    v0 = 0.0
    v1 = 0.0
    for t in range(n):
        xt = 1.0 if t == 0 else 0.0
        out = xt * b0 + v0
        v0_new = xt * b1 + v1 - out * a1
        v1_new = xt * b2 - out * a2
        v0, v1 = v0_new, v1_new
        g[t] = out
    return g


def _coef_block(g, kb):
    """[128, 128 + kb] moving operand: columns = output offset i.

    A^T[j, i] = g[i - j]          (within-chunk taps, i in [0,128))
    B^T[j, i] = g[i + 128 - j]    (taps reaching one chunk back, i in [0,kb))
    """
    K = len(g)
    A = np.zeros((_P, _P), dtype=np.float64)
    Bm = np.zeros((_P, kb), dtype=np.float64)
    for j in range(_P):
        for i in range(_P):
            if 0 <= i - j < K:
                A[j, i] = g[i - j]
        for i in range(kb):
            k = i + _P - j
            if 0 <= k < K:
                Bm[j, i] = g[k]
    return np.concatenate([A, Bm], axis=1).astype(np.float32)


def _build_program(mode, kb, repeat=1):
    from concourse import bacc, mybir, tile

    nc = bacc.Bacc("TRN2", target_bir_lowering=False, debug=False)
    f32 = mybir.dt.float32
    cdt = mybir.dt.float32r if mode == "f32r" else f32

    NC = _P + kb                      # moving operand width
    SLOT = 256 if NC <= 256 else 512  # psum slot stride (bank-crossing safe)
    x_d = nc.dram_tensor("x", [_S, _P, _F], f32, kind="ExternalInput")
    coef_d = nc.dram_tensor("coef", [_P, NC], f32, kind="ExternalInput")
    id_d = nc.dram_tensor("ident", [_P, _P], f32, kind="ExternalInput")
    y_d = nc.dram_tensor("y", [_S, _P, _F], f32, kind="ExternalOutput")

    XT_SLOTS = _NT + 1                # 16 transposed tiles + shifted m1 tile

    with tile.TileContext(nc) as tc, ExitStack() as ctx:
        cpool = ctx.enter_context(tc.tile_pool(name="consts", bufs=1))
        xpool = ctx.enter_context(tc.tile_pool(name="xin", bufs=2))
        xtpool = ctx.enter_context(tc.tile_pool(name="xt", bufs=2))
        ypool = ctx.enter_context(tc.tile_pool(name="yout", bufs=2))
        ptp = ctx.enter_context(tc.tile_pool(name="pt", bufs=2, space="PSUM"))
        pyp = ctx.enter_context(tc.tile_pool(name="py", bufs=3, space="PSUM"))

        id_sb = cpool.tile([_P, _P], f32)
        nc.sync.dma_start(id_sb[:], id_d.ap())
        coef_sb = cpool.tile([_P, NC], f32)
        nc.sync.dma_start(coef_sb[:], coef_d.ap())
        if mode == "f32r":
            coef_c = cpool.tile([_P, NC], cdt)
            nc.vector.tensor_copy(coef_c[:], coef_sb[:])
        else:
            coef_c = coef_sb

        from contextlib import nullcontext
        loop_ctx = tc.For_i(0, repeat, 1) if repeat > 1 else nullcontext()
        with loop_ctx:
          for s in range(_S):
            # --- load x[s] as [128, 2048], partition p = x[p*2048 + f] ---
            xs = xpool.tile([_P, _F], f32)
            for q in range(4):
                nc.sync.dma_start(
                    xs[:, q * 512 : (q + 1) * 512],
                    x_d.ap()[s][:, q * 512 : (q + 1) * 512],
                )

            # --- PE transposes, 4 per PSUM bank, ACT evacuation ---
            # Tile 15 is transposed first so the m1 boundary tile (and its
            # matmul) can run early, keeping all PSUM pairs short-lived.
            perm = [15] + list(range(15))
            xt = xtpool.tile([_P, XT_SLOTS * _P], cdt)
            xt32 = xt[:].bitcast(f32)
            for gidx in range(4):
                ptile = ptp.tile([_P, 512], f32)
                grp = perm[4 * gidx : 4 * gidx + 4]
                for q, f1 in enumerate(grp):
                    nc.tensor.transpose(
                        ptile[:, q * _P : (q + 1) * _P],
                        xs[:, f1 * _P : (f1 + 1) * _P],
                        id_sb[:],
                    )
                # copy contiguous runs of the permuted group into xt
                q0 = 0
                while q0 < 4:
                    q1 = q0 + 1
                    while q1 < 4 and grp[q1] == grp[q1 - 1] + 1:
                        q1 += 1
                    nc.scalar.copy(
                        xt[:, grp[q0] * _P : (grp[q0] + q1 - q0) * _P],
                        ptile[:, q0 * _P : q1 * _P],
                    )
                    q0 = q1

            # --- m1 boundary tile: m1[col p] = tile15[col p-1], col 0 = 0 ---
            m1 = _NT * _P
            nc.gpsimd.memset(xt32[:, m1 : m1 + 1], 0.0)
            nc.gpsimd.tensor_copy(
                xt32[:, m1 + 1 : m1 + _P],
                xt32[:, 15 * _P : 16 * _P - 1],
            )

            # --- matmuls (fused [A|B] moving operand) + batched evacuation ---
            # Groups of 4 matmuls per PSUM tile:
            #   G0=[m1,t0,t1,t2] G1=[t3..6] G2=[t7..10] G3=[t11..14] G4=[t15]
            # ys[f1] = A-half(f1) + B-half(f1-1); m1's B-half feeds ys[0].
            ys = ypool.tile([_P, _F], f32)
            groups = [[_NT, 0, 1, 2], [3, 4, 5, 6], [7, 8, 9, 10],
                      [11, 12, 13, 14], [15]]
            ptiles = []

            def mm_group(gi):
                grp = groups[gi]
                pt_ = pyp.tile([_P, 4 * SLOT], f32, tag="py")
                for k, idx in enumerate(grp):
                    nc.tensor.matmul(
                        pt_[:, k * SLOT : k * SLOT + NC],
                        xt[:, idx * _P : (idx + 1) * _P],
                        coef_c[:],
                        start=True,
                        stop=True,
                    )
                ptiles.append(pt_)

            def a_copy(gi, eng):
                # copy A-halves of the group's data tiles (skip m1) into ys
                grp = groups[gi]
                pt_ = ptiles[gi]
                k0 = 1 if gi == 0 else 0
                n = len(grp) - k0
                t0 = grp[k0]
                src = pt_[:, k0 * SLOT : (k0 + n) * SLOT].rearrange(
                    "p (n w) -> p n w", w=SLOT
                )[:, :, 0:_P]
                dst = ys[:, t0 * _P : (t0 + n) * _P].rearrange(
                    "p (n w) -> p n w", w=_P
                )
                eng(dst, src)

            def b_add(gi):
                # ys[f1] += B-half(f1-1): group gi's slots feed the next tiles
                grp = groups[gi]
                if grp[-1] == 15:
                    grp = grp[:-1]      # tile 15's B-half is discarded
                    if not grp:
                        return
                pt_ = ptiles[gi]
                n = len(grp)
                tdst = 0 if gi == 0 else groups[gi][0] + 1
                src = pt_[:, 0 : n * SLOT].rearrange(
                    "p (n w) -> p n w", w=SLOT
                )[:, :, _P : _P + kb]
                dst = ys[:, tdst * _P : (tdst + n) * _P].rearrange(
                    "p (n w) -> p n w", w=_P
                )[:, :, 0:kb]
                nc.vector.tensor_add(dst, src, dst)

            act_copy = nc.scalar.copy
            dve_copy = nc.vector.tensor_copy
            # A-copy engine per group: balance ACT (also does Xt copies)
            a_eng = [dve_copy, act_copy, dve_copy, act_copy, dve_copy]

            mm_group(0)
            a_copy(0, a_eng[0])
            for gi in range(1, 5):
                mm_group(gi)
                a_copy(gi, a_eng[gi])
                b_add(gi - 1)
            b_add(4)

            nc.gpsimd.dma_start(y_d.ap()[s], ys[:])

    nc.compile()
    return nc


def _make_runner(mode, kb, repeat=1):
    """Compile the bass program and wrap it in a cached shard_map'd jit."""
    import jax
    import numpy as _np
    from jax.sharding import Mesh, PartitionSpec
    from jax.experimental.shard_map import shard_map
    from concourse import bass2jax, mybir

    nc = _build_program(mode, kb, repeat)

    if os.environ.get("BIQUAD_SIM") == "1":
        def run_sim(x_all, coef):
            from concourse import bass_interp
            y_all = np.zeros_like(x_all)
            ident = np.eye(_P, dtype=np.float32)
            ncs = int(os.environ.get("BIQUAD_SIM_CORES", str(_NCORES)))
            for c in range(ncs):
                sim = bass_interp.CoreSim(nc)
                sim.tensor("x")[:] = x_all[c * _S : (c + 1) * _S]
                sim.tensor("coef")[:] = coef
                sim.tensor("ident")[:] = ident
                sim.simulate()
                y_all[c * _S : (c + 1) * _S] = sim.tensor("y")
            return y_all
        return run_sim

    bass2jax.install_neuronx_cc_hook()

    partition_name = (
        nc.partition_id_tensor.name if nc.partition_id_tensor else None
    )
    in_names, out_names, out_avals = [], [], []
    for alloc in nc.m.functions[0].allocations:
        if not isinstance(alloc, mybir.MemoryLocationSet):
            continue
        name = alloc.memorylocations[0].name
        if alloc.kind == "ExternalInput":
            if name != partition_name:
                in_names.append(name)
        elif alloc.kind == "ExternalOutput":
            out_names.append(name)
            out_avals.append(
                jax.core.ShapedArray(
                    tuple(alloc.tensor_shape), mybir.dt.np(alloc.dtype)
                )
            )
    n_params = len(in_names)
    in_names.extend(out_names)
    if partition_name is not None:
        in_names.append(partition_name)

    def _body(*args):
        operands = list(args)
        if partition_name is not None:
            operands.append(bass2jax.partition_id_tensor())
        outs = bass2jax._bass_exec_p.bind(
            *operands,
            out_avals=tuple(out_avals),
            in_names=tuple(in_names),
            out_names=tuple(out_names),
            lowering_input_output_aliases=(),
            sim_require_finite=True,
            sim_require_nnan=True,
            nc=nc,
        )
        return tuple(outs)

    devices = jax.devices()[:_NCORES]
    mesh = Mesh(_np.asarray(devices), ("core",))
    n_outs = len(out_names)
    in_specs = (PartitionSpec("core"),) * (n_params + n_outs)
    out_specs = (PartitionSpec("core"),) * n_outs
    sharded = jax.jit(
        shard_map(
            _body, mesh=mesh, in_specs=in_specs, out_specs=out_specs,
            check_rep=False,
        ),
        keep_unused=True,
    )

    name_to_idx = {n: i for i, n in enumerate(in_names[:n_params])}
    ident = np.eye(_P, dtype=np.float32)

    def run_hw(x_all, coef):
        # x_all: [64, 128, 2048] fp32; returns y_all same shape
        per_core_ins = {
            "x": x_all.reshape(_NCORES * _S, _P, _F),
            "coef": np.concatenate([coef] * _NCORES, axis=0),
            "ident": np.concatenate([ident] * _NCORES, axis=0),
        }
        args = [None] * n_params
        for n, i in name_to_idx.items():
            args[i] = per_core_ins[n]
        zeros = [
            np.zeros((_NCORES * a.shape[0], *a.shape[1:]), a.dtype)
            for a in out_avals
        ]
        outs = sharded(*args, *zeros)
        y_idx = out_names.index("y")
        return np.asarray(outs[y_idx]).reshape(_B, _P, _F)

    run_hw.sharded = sharded
    run_hw.meta = (in_names, out_names, out_avals, n_params, name_to_idx, ident)
    run_hw.nc = nc

    def make_chain():
        """Jit that runs the kernel k (runtime scalar) times back-to-back on
        device, feeding y back as x — for timing (marginal cost per step ≈
        one on-device execution). fori_loop keeps the bass_exec custom call
        appearing exactly once in the module (hook limitation), and a
        runtime k avoids recompiling per chain length."""
        x_idx = name_to_idx["x"]
        y_idx = out_names.index("y")

        def chained(k, *args):
            args = list(args)

            def body(_, x):
                a = list(args)
                a[x_idx] = x
                return _body(*a)[y_idx]

            y = jax.lax.fori_loop(0, k, body, args[x_idx])
            return (y,)

        return jax.jit(
            shard_map(
                chained, mesh=mesh,
                in_specs=(PartitionSpec(),) + in_specs,
                out_specs=(PartitionSpec("core"),),
                check_rep=False,
            ),
            keep_unused=True,
        )

    run_hw.make_chain = make_chain
    return run_hw


def _get_runner(mode, kb, repeat=1):
    key = (mode, kb, repeat, os.environ.get("BIQUAD_SIM") == "1")
    if key not in _runner_cache:
        _runner_cache[key] = _make_runner(mode, kb, repeat)
    return _runner_cache[key]


def _prepare(b0, b1, b2, a1, a2):
    """Impulse response, truncation length, coefficient block."""
    g = _impulse_response(b0, b1, b2, a1, a2, 2 * _P)
    mag = np.abs(g)
    scale = mag.max() + 1e-300
    sig = np.nonzero(mag > 1e-9 * scale)[0]
    K = int(sig[-1]) + 1 if len(sig) else 1
    if K > _P:
        raise ValueError(
            f"impulse response needs {K} taps (> {_P}); filter too close "
            "to instability for the truncated-FIR kernel"
        )
    kb = max(32, ((K + 31) // 32) * 32)   # B-half width, 32-col aligned
    if _MODE == "f32r":
        kb = _P                            # keep N >= 256 for full-rate f32r
    coef = _coef_block(g[: _P + kb], kb)
    return coef, kb


def kernel(x, b0, b1, b2, a1, a2):
    assert x.shape == (_B, _T, 1), x.shape
    coef, kb = _prepare(
        float(b0[0]), float(b1[0]), float(b2[0]), float(a1[0]), float(a2[0])
    )
    run = _get_runner(_MODE, kb)
    x_all = np.ascontiguousarray(x, dtype=np.float32).reshape(_B, _P, _F)
    y_all = run(x_all, coef)
    return y_all.reshape(_B, _T, 1)


# revision 12
# speedup vs baseline: 1402.0543x; 4.1551x over previous
"""Direct-Form-II biquad (order-2 IIR) over [B=64, T=262144, 1] on 8 trn2 cores.

Algorithm
---------
The recurrence
    y[t] = b0 x[t] + b1 x[t-1] + b2 x[t-2] - a1 y[t-1] - a2 y[t-2]
is a linear time-invariant filter whose impulse response g decays
geometrically (|poles| < 1 for the sampled coefficients), so to fp32
precision the IIR equals a short FIR: y = conv(x, g[:K]).

On device the FIR is computed with the tensor engine in overlap-save form.
Per sequence, x is laid out in SBUF as [128 partitions, 2048] with partition
p holding x[p*2048 : (p+1)*2048] (contiguous DMA). Each 128x128 tile of that
layout holds 128 chunks (partitions = chunk index c = p*16 + f1, free =
within-chunk time j). Tiles are PE-transposed so j lands on partitions, then
one matmul per tile, with the transposed tile as the stationary operand and a
fused [A^T | B^T] Toeplitz coefficient block as the moving operand, produces
the within-chunk FIR term (A-half) and the spill-over into the next chunk
(B-half). ys[f1] = A(f1) + B(f1-1) is assembled during PSUM evacuation:
an A-copy (ACT/DVE) plus a read-modify-write B-add (DVE) — PSUM has a single
DVE read port, so the two PSUM halves are never read by one instruction.

Sharding: pure data parallelism, batch 64 -> 8 sequences per core.
"""

import os
from contextlib import ExitStack

import numpy as np

_B, _T = 64, 262144
_NCORES = 8
_S = _B // _NCORES          # sequences per core
_P = 128                    # partitions / chunk length
_F = _T // _P               # 2048 free columns per sequence
_NT = _F // _P              # 16 tiles per sequence

# 'fp32'  : exact fp32 matmuls (4 cycles/row on PE)
# 'f32r'  : rounded fp32 (12-bit mantissa) matmuls at full PE rate
_MODE = os.environ.get("BIQUAD_MODE", "fp32")

_runner_cache = {}


def _impulse_response(b0, b1, b2, a1, a2, n):
    """Float64 impulse response of the reference recurrence."""
    g = np.zeros(n, dtype=np.float64)
    v0 = 0.0
    v1 = 0.0
    for t in range(n):
        xt = 1.0 if t == 0 else 0.0
        out = xt * b0 + v0
        v0_new = xt * b1 + v1 - out * a1
        v1_new = xt * b2 - out * a2
        v0, v1 = v0_new, v1_new
        g[t] = out
    return g


def _coef_block(g, kb):
    """[128, 128 + kb] moving operand: columns = output offset i.

    A^T[j, i] = g[i - j]          (within-chunk taps, i in [0,128))
    B^T[j, i] = g[i + 128 - j]    (taps reaching one chunk back, i in [0,kb))
    """
    K = len(g)
    A = np.zeros((_P, _P), dtype=np.float64)
    Bm = np.zeros((_P, kb), dtype=np.float64)
    for j in range(_P):
        for i in range(_P):
            if 0 <= i - j < K:
                A[j, i] = g[i - j]
        for i in range(kb):
            k = i + _P - j
            if 0 <= k < K:
                Bm[j, i] = g[k]
    return np.concatenate([A, Bm], axis=1).astype(np.float32)


def _build_program(mode, kb, repeat=1):
    from concourse import bacc, mybir, tile

    nc = bacc.Bacc("TRN2", target_bir_lowering=False, debug=False)
    f32 = mybir.dt.float32
    cdt = mybir.dt.float32r if mode == "f32r" else f32

    NC = _P + kb                      # moving operand width
    SLOT = 256 if NC <= 256 else 512  # psum slot stride (bank-crossing safe)
    x_d = nc.dram_tensor("x", [_S, _P, _F], f32, kind="ExternalInput")
    coef_d = nc.dram_tensor("coef", [_P, NC], f32, kind="ExternalInput")
    id_d = nc.dram_tensor("ident", [_P, _P], f32, kind="ExternalInput")
    y_d = nc.dram_tensor("y", [_S, _P, _F], f32, kind="ExternalOutput")

    XT_SLOTS = _NT + 1                # 16 transposed tiles + shifted m1 tile

    with tile.TileContext(nc) as tc, ExitStack() as ctx:
        cpool = ctx.enter_context(tc.tile_pool(name="consts", bufs=1))
        xpool = ctx.enter_context(tc.tile_pool(name="xin", bufs=2))
        xtpool = ctx.enter_context(tc.tile_pool(name="xt", bufs=2))
        ypool = ctx.enter_context(tc.tile_pool(name="yout", bufs=2))
        ptp = ctx.enter_context(tc.tile_pool(name="pt", bufs=2, space="PSUM"))
        pyp = ctx.enter_context(tc.tile_pool(name="py", bufs=3, space="PSUM"))

        id_sb = cpool.tile([_P, _P], f32)
        nc.sync.dma_start(id_sb[:], id_d.ap())
        coef_sb = cpool.tile([_P, NC], f32)
        nc.sync.dma_start(coef_sb[:], coef_d.ap())
        if mode == "f32r":
            coef_c = cpool.tile([_P, NC], cdt)
            nc.vector.tensor_copy(coef_c[:], coef_sb[:])
        else:
            coef_c = coef_sb

        from contextlib import nullcontext
        loop_ctx = tc.For_i(0, repeat, 1) if repeat > 1 else nullcontext()
        with loop_ctx:
          for s in range(_S):
            # --- load x[s] as [128, 2048], partition p = x[p*2048 + f] ---
            # one 1 MiB dma_start (max-bandwidth shape: 128 partitions, >=1MiB)
            xs = xpool.tile([_P, _F], f32)
            nc.sync.dma_start(xs[:], x_d.ap()[s])

            # --- PE transposes, 4 per PSUM bank, ACT evacuation ---
            # Tile 15 is transposed first so the m1 boundary tile (and its
            # matmul) can run early, keeping all PSUM pairs short-lived.
            perm = [15] + list(range(15))
            xt = xtpool.tile([_P, XT_SLOTS * _P], cdt)
            xt32 = xt[:].bitcast(f32)
            for gidx in range(4):
                ptile = ptp.tile([_P, 512], f32)
                grp = perm[4 * gidx : 4 * gidx + 4]
                for q, f1 in enumerate(grp):
                    nc.tensor.transpose(
                        ptile[:, q * _P : (q + 1) * _P],
                        xs[:, f1 * _P : (f1 + 1) * _P],
                        id_sb[:],
                    )
                # copy contiguous runs of the permuted group into xt
                q0 = 0
                while q0 < 4:
                    q1 = q0 + 1
                    while q1 < 4 and grp[q1] == grp[q1 - 1] + 1:
                        q1 += 1
                    nc.scalar.copy(
                        xt[:, grp[q0] * _P : (grp[q0] + q1 - q0) * _P],
                        ptile[:, q0 * _P : q1 * _P],
                    )
                    q0 = q1

            # --- m1 boundary tile: m1[col p] = tile15[col p-1], col 0 = 0 ---
            m1 = _NT * _P
            nc.gpsimd.memset(xt32[:, m1 : m1 + 1], 0.0)
            nc.gpsimd.tensor_copy(
                xt32[:, m1 + 1 : m1 + _P],
                xt32[:, 15 * _P : 16 * _P - 1],
            )

            # --- matmuls (fused [A|B] moving operand) + batched evacuation ---
            # Groups of 4 matmuls per PSUM tile:
            #   G0=[m1,t0,t1,t2] G1=[t3..6] G2=[t7..10] G3=[t11..14] G4=[t15]
            # ys[f1] = A-half(f1) + B-half(f1-1); m1's B-half feeds ys[0].
            ys = ypool.tile([_P, _F], f32)
            groups = [[_NT, 0, 1, 2], [3, 4, 5, 6], [7, 8, 9, 10],
                      [11, 12, 13, 14], [15]]
            ptiles = []

            def mm_group(gi):
                grp = groups[gi]
                pt_ = pyp.tile([_P, 4 * SLOT], f32, tag="py")
                for k, idx in enumerate(grp):
                    nc.tensor.matmul(
                        pt_[:, k * SLOT : k * SLOT + NC],
                        xt[:, idx * _P : (idx + 1) * _P],
                        coef_c[:],
                        start=True,
                        stop=True,
                    )
                ptiles.append(pt_)

            def a_copy(gi, eng):
                # copy A-halves of the group's data tiles (skip m1) into ys
                grp = groups[gi]
                pt_ = ptiles[gi]
                k0 = 1 if gi == 0 else 0
                n = len(grp) - k0
                t0 = grp[k0]
                src = pt_[:, k0 * SLOT : (k0 + n) * SLOT].rearrange(
                    "p (n w) -> p n w", w=SLOT
                )[:, :, 0:_P]
                dst = ys[:, t0 * _P : (t0 + n) * _P].rearrange(
                    "p (n w) -> p n w", w=_P
                )
                eng(dst, src)

            def b_add(gi):
                # ys[f1] += B-half(f1-1): group gi's slots feed the next tiles
                grp = groups[gi]
                if grp[-1] == 15:
                    grp = grp[:-1]      # tile 15's B-half is discarded
                    if not grp:
                        return
                pt_ = ptiles[gi]
                n = len(grp)
                tdst = 0 if gi == 0 else groups[gi][0] + 1
                src = pt_[:, 0 : n * SLOT].rearrange(
                    "p (n w) -> p n w", w=SLOT
                )[:, :, _P : _P + kb]
                dst = ys[:, tdst * _P : (tdst + n) * _P].rearrange(
                    "p (n w) -> p n w", w=_P
                )[:, :, 0:kb]
                nc.vector.tensor_add(dst, src, dst)

            act_copy = nc.scalar.copy
            dve_copy = nc.vector.tensor_copy
            # A-copy engine per group: balance ACT (also does Xt copies)
            a_eng = [dve_copy, act_copy, dve_copy, act_copy, dve_copy]

            mm_group(0)
            a_copy(0, a_eng[0])
            for gi in range(1, 5):
                mm_group(gi)
                a_copy(gi, a_eng[gi])
                b_add(gi - 1)
            b_add(4)

            nc.gpsimd.dma_start(y_d.ap()[s], ys[:])

    nc.compile()
    return nc


def _make_runner(mode, kb, repeat=1):
    """Compile the bass program and wrap it in a cached shard_map'd jit."""
    import jax
    import numpy as _np
    from jax.sharding import Mesh, PartitionSpec
    from jax.experimental.shard_map import shard_map
    from concourse import bass2jax, mybir

    nc = _build_program(mode, kb, repeat)

    if os.environ.get("BIQUAD_SIM") == "1":
        def run_sim(x_all, coef):
            from concourse import bass_interp
            y_all = np.zeros_like(x_all)
            ident = np.eye(_P, dtype=np.float32)
            ncs = int(os.environ.get("BIQUAD_SIM_CORES", str(_NCORES)))
            for c in range(ncs):
                sim = bass_interp.CoreSim(nc)
                sim.tensor("x")[:] = x_all[c * _S : (c + 1) * _S]
                sim.tensor("coef")[:] = coef
                sim.tensor("ident")[:] = ident
                sim.simulate()
                y_all[c * _S : (c + 1) * _S] = sim.tensor("y")
            return y_all
        return run_sim

    bass2jax.install_neuronx_cc_hook()

    partition_name = (
        nc.partition_id_tensor.name if nc.partition_id_tensor else None
    )
    in_names, out_names, out_avals = [], [], []
    for alloc in nc.m.functions[0].allocations:
        if not isinstance(alloc, mybir.MemoryLocationSet):
            continue
        name = alloc.memorylocations[0].name
        if alloc.kind == "ExternalInput":
            if name != partition_name:
                in_names.append(name)
        elif alloc.kind == "ExternalOutput":
            out_names.append(name)
            out_avals.append(
                jax.core.ShapedArray(
                    tuple(alloc.tensor_shape), mybir.dt.np(alloc.dtype)
                )
            )
    n_params = len(in_names)
    in_names.extend(out_names)
    if partition_name is not None:
        in_names.append(partition_name)

    def _body(*args):
        operands = list(args)
        if partition_name is not None:
            operands.append(bass2jax.partition_id_tensor())
        outs = bass2jax._bass_exec_p.bind(
            *operands,
            out_avals=tuple(out_avals),
            in_names=tuple(in_names),
            out_names=tuple(out_names),
            lowering_input_output_aliases=(),
            sim_require_finite=True,
            sim_require_nnan=True,
            nc=nc,
        )
        return tuple(outs)

    devices = jax.devices()[:_NCORES]
    mesh = Mesh(_np.asarray(devices), ("core",))
    n_outs = len(out_names)
    in_specs = (PartitionSpec("core"),) * (n_params + n_outs)
    out_specs = (PartitionSpec("core"),) * n_outs
    sharded = jax.jit(
        shard_map(
            _body, mesh=mesh, in_specs=in_specs, out_specs=out_specs,
            check_rep=False,
        ),
        keep_unused=True,
    )

    name_to_idx = {n: i for i, n in enumerate(in_names[:n_params])}
    ident = np.eye(_P, dtype=np.float32)

    def run_hw(x_all, coef):
        # x_all: [64, 128, 2048] fp32; returns y_all same shape
        per_core_ins = {
            "x": x_all.reshape(_NCORES * _S, _P, _F),
            "coef": np.concatenate([coef] * _NCORES, axis=0),
            "ident": np.concatenate([ident] * _NCORES, axis=0),
        }
        args = [None] * n_params
        for n, i in name_to_idx.items():
            args[i] = per_core_ins[n]
        zeros = [
            np.zeros((_NCORES * a.shape[0], *a.shape[1:]), a.dtype)
            for a in out_avals
        ]
        outs = sharded(*args, *zeros)
        y_idx = out_names.index("y")
        return np.asarray(outs[y_idx]).reshape(_B, _P, _F)

    run_hw.sharded = sharded
    run_hw.meta = (in_names, out_names, out_avals, n_params, name_to_idx, ident)
    run_hw.nc = nc

    def make_chain():
        """Jit that runs the kernel k (runtime scalar) times back-to-back on
        device, feeding y back as x — for timing (marginal cost per step ≈
        one on-device execution). fori_loop keeps the bass_exec custom call
        appearing exactly once in the module (hook limitation), and a
        runtime k avoids recompiling per chain length."""
        x_idx = name_to_idx["x"]
        y_idx = out_names.index("y")

        def chained(k, *args):
            args = list(args)

            def body(_, x):
                a = list(args)
                a[x_idx] = x
                return _body(*a)[y_idx]

            y = jax.lax.fori_loop(0, k, body, args[x_idx])
            return (y,)

        return jax.jit(
            shard_map(
                chained, mesh=mesh,
                in_specs=(PartitionSpec(),) + in_specs,
                out_specs=(PartitionSpec("core"),),
                check_rep=False,
            ),
            keep_unused=True,
        )

    run_hw.make_chain = make_chain
    return run_hw


def _get_runner(mode, kb, repeat=1):
    key = (mode, kb, repeat, os.environ.get("BIQUAD_SIM") == "1")
    if key not in _runner_cache:
        _runner_cache[key] = _make_runner(mode, kb, repeat)
    return _runner_cache[key]


def _prepare(b0, b1, b2, a1, a2):
    """Impulse response, truncation length, coefficient block."""
    g = _impulse_response(b0, b1, b2, a1, a2, 2 * _P)
    mag = np.abs(g)
    scale = mag.max() + 1e-300
    sig = np.nonzero(mag > 1e-9 * scale)[0]
    K = int(sig[-1]) + 1 if len(sig) else 1
    if K > _P:
        raise ValueError(
            f"impulse response needs {K} taps (> {_P}); filter too close "
            "to instability for the truncated-FIR kernel"
        )
    kb = max(32, ((K + 15) // 16) * 16)   # B-half width, 16-col aligned
    if _MODE == "f32r":
        kb = _P                            # keep N >= 256 for full-rate f32r
    coef = _coef_block(g[: _P + kb], kb)
    return coef, kb


def kernel(x, b0, b1, b2, a1, a2):
    assert x.shape == (_B, _T, 1), x.shape
    coef, kb = _prepare(
        float(b0[0]), float(b1[0]), float(b2[0]), float(a1[0]), float(a2[0])
    )
    run = _get_runner(_MODE, kb)
    x_all = np.ascontiguousarray(x, dtype=np.float32).reshape(_B, _P, _F)
    y_all = run(x_all, coef)
    return y_all.reshape(_B, _T, 1)
